# revision 38
# baseline (speedup 1.0000x reference)
"""Position Attention Module (DANet) on 8 Trainium2 NeuronCores.

Reference computation (per batch b of 4):
  xf = x[b] : [C=512, N=4096]
  q = Wq@xf + bq : [64, N];  k = Wk@xf + bk : [64, N];  v = Wv@xf + bv : [512, N]
  scores[i,j] = q[:,i].k[:,j];  attn = softmax_j(scores)
  out[c,i] = alpha * sum_j v[c,j] attn[i,j]

Sharding: 2 cores per batch, each core owns half the query rows (i), full k.
Per-core x is pre-rolled on host so the owned i-half is always columns 0:2048.

Device design (v2):
  - alpha folded into Wv/bv on the host; V path reassociated:
    out = Wv@(x@attnT)+bv so the 512x512 projection runs on the 2048
    attention-averaged columns and v never materializes.
  - fp16 q/k path, bf16 AV path, exp without max-subtraction (|s|max ~56 < 88).
  - scores matmul contraction is only CQ=64, so two score j-tiles run
    CONCURRENTLY in different PE row groups (tile_position (0,0)/(64,0)):
    k lives in a parity layout k2[0:64]=even j-tiles, k2[64:128]=odd, with q
    duplicated in both partition halves. Halves the scores' PE time.
  - engine budget per j-step (inner loop): PE 4xAV + score-pair share;
    ACT does the exp + outproj bias; DVE does the softmax-denominator
    accumulation, yps evictions, reciprocal and outproj normalize. The
    Pool engine is kept OUT of the datapath: its elementwise ops are slow
    (tensor_scalar ~8us/tile) and its shared SBUF port slows concurrent
    DVE ops severely.
  - score-pair lookahead is 2 j-steps: the 4-bank score PSUM pool then has
    a full cycle of WAR slack (at lookahead 4 every pair matmul waited on
    an exp completion + ~100ns semaphore delay).
  - DMA need-order across both HWDGE queues; the scalar queue only carries
    the few earliest transfers so a full DMA ring can never head-block the
    evictions/exps behind the trigger on the ACT stream; sync (no compute
    role) takes the long stream. qk projection of the second x half is
    folded into i-tile 0's j-loop so the input stream hides under compute.
  - boundary: next i-tile's first score pair is emitted BEFORE the
    denominator work; yps freed by 4 DVE copies before the deferred last
    denominator add + ones-matmul + reciprocal, so the next i-tile's AV
    reuses the PSUM banks after ~0.2us instead of ~1us.
  - known floor: ~150us of PE streaming + ~8us preamble + ~8.5us teardown;
    the 64-row score LDWEIGHTS cannot use the PE background weight buffer,
    so each pair<->AV weight-set transition costs ~100ns unhidden.
"""
import numpy as np
import ml_dtypes


B, C, HW = 4, 512, 4096
CQ = 64
NCORES = 8
IH = HW // 2          # 2048 query rows per core
ITILE = 512           # i-tile (psum free dim)
NITILES = IH // ITILE # 4
JT = 128              # j-tile (128 keys per scores tile)
NJT = HW // JT        # 32
XB = 2048             # x/xt DMA block columns
NCC = C // 128        # 4 contraction chunks of 128 over C

_cache = {}


def _build():
    import concourse.bacc as bacc
    import concourse.tile as tile
    import concourse.mybir as mybir
    from concourse.bass_utils import run_bass_kernel_spmd

    f32 = mybir.dt.float32
    f32r = mybir.dt.float32r
    bf16 = mybir.dt.bfloat16
    fp16 = mybir.dt.float16
    AF = mybir.ActivationFunctionType

    nc = bacc.Bacc("TRN2", target_bir_lowering=False, debug=False)

    x_d = nc.dram_tensor("x", [C, HW], fp16, kind="ExternalInput")
    # xt packed j-tile-major: col block t*C..(t+1)*C row p holds xT[t*128+p, :]
    xt_d = nc.dram_tensor("xt", [128, NJT * C], bf16, kind="ExternalInput")
    wqk_d = nc.dram_tensor("wqk", [128, NCC * 2 * CQ], fp16, kind="ExternalInput")
    wvt_d = nc.dram_tensor("wvt", [128, NCC * C], bf16, kind="ExternalInput")
    # bias cols: 0=[bk;bk] 1=[bq;bq] 2..5=bv chunks (alpha folded)
    bias_d = nc.dram_tensor("bias", [128, 6], f32, kind="ExternalInput")
    out_d = nc.dram_tensor("out", [C, IH], f32, kind="ExternalOutput")

    with tile.TileContext(nc) as tc:
        with (
            tc.tile_pool(name="const", bufs=1) as cpool,
            tc.tile_pool(name="kq", bufs=1) as kqpool,
            tc.tile_pool(name="xt", bufs=1) as xtpool,
            tc.tile_pool(name="xin", bufs=16) as xpool,
            tc.tile_pool(name="expp", bufs=8) as epool,
            tc.tile_pool(name="dnm", bufs=2) as dpool,
            tc.tile_pool(name="ysb", bufs=8) as ypool,
            tc.tile_pool(name="ost", bufs=4) as opool,
            tc.tile_pool(name="rows", bufs=2) as rpool,
            tc.tile_pool(name="psc", bufs=4, space="PSUM") as psc,
            tc.tile_pool(name="py", bufs=4, space="PSUM") as py,
        ):
            # ---------------- static tiles ----------------
            wqk_c = cpool.tile([128, NCC * 2 * CQ], fp16, tag="wqk")
            wvt_c = cpool.tile([128, NCC * C], bf16, tag="wvt")
            bias_c = cpool.tile([128, 6], f32, tag="bias")
            ones_f = cpool.tile([128, 128], f32, tag="onesf")
            ones_sq = cpool.tile([128, 128], f32r, tag="onessq")
            warm = cpool.tile([128, 1], f32, tag="warm")
            # parity layouts: k2[0:64]=even j-tiles, k2[64:128]=odd; q dup'd
            k2 = kqpool.tile([128, (NJT // 2) * JT], fp16, tag="k2")
            q2 = kqpool.tile([128, IH], fp16, tag="q2")
            xtw = [xtpool.tile([128, XB], bf16, tag=f"xt{w}", name=f"xt{w}")
                   for w in range((NJT * C) // XB)]
            # x in [128, 1024] half-blocks: xh[hh][cc] covers cols hh*1024..
            xh = [[None] * NCC for _ in range(HW // 1024)]

            # ---------------- DMA triggers (need-order) ----------
            # scalar (ACT) carries only the few earliest transfers so its
            # trigger stream never ring-blocks the evictions/exps behind it;
            # sync has no compute role, so it takes the long stream.
            def dma_xh(hh, qs, split=False):
                for cc in range(NCC):
                    t = xpool.tile([128, 1024], fp16, tag="x", name=f"x{hh}_{cc}")
                    csl = slice(cc * 128, (cc + 1) * 128)
                    if split:  # halves so the first qk h-block starts sooner
                        qs[cc % len(qs)].dma_start(
                            t[:, 0:512], x_d[csl, hh * 1024:hh * 1024 + 512])
                    xh[hh][cc] = t
                if split:
                    for cc in range(NCC):
                        csl = slice(cc * 128, (cc + 1) * 128)
                        qs[cc % len(qs)].dma_start(
                            xh[hh][cc][:, 512:1024],
                            x_d[csl, hh * 1024 + 512:(hh + 1) * 1024])
                else:
                    for cc in range(NCC):
                        csl = slice(cc * 128, (cc + 1) * 128)
                        qs[cc % len(qs)].dma_start(
                            xh[hh][cc][:], x_d[csl, hh * 1024:(hh + 1) * 1024])

            nc.sync.dma_start(wqk_c[:], wqk_d[:])
            nc.scalar.dma_start(bias_c[:], bias_d[:])
            dma_xh(0, [nc.sync, nc.scalar], split=True)
            nc.scalar.dma_start(xtw[0][:], xt_d[:, 0:XB])
            dma_xh(1, [nc.sync, nc.scalar])
            nc.scalar.dma_start(xtw[1][:], xt_d[:, XB:2 * XB])
            dma_xh(2, [nc.sync])
            nc.sync.dma_start(xtw[2][:], xt_d[:, 2 * XB:3 * XB])
            nc.scalar.dma_start(xtw[3][:], xt_d[:, 3 * XB:4 * XB])
            dma_xh(3, [nc.sync, nc.scalar])
            for w in range(4, 8):
                nc.sync.dma_start(xtw[w][:], xt_d[:, w * XB:(w + 1) * XB])
            nc.sync.dma_start(wvt_c[:], wvt_d[:])

            # constants + ACT table warm-up (overlaps the DMA wait)
            nc.gpsimd.memset(ones_f[:], 1.0)
            nc.gpsimd.tensor_copy(ones_sq[:], ones_f[:])
            nc.scalar.activation(warm[:], ones_f[:, 0:1], AF.Exp)

            # ---------------- q/k projections ----------------
            def emit_qk(w, hs=(0, 4)):
                # x block w covers j-tiles w*16 .. w*16+15; q only from w==0
                nq = 2 * CQ if w == 0 else CQ
                for h in range(*hs):
                    hsl = slice((h % 2) * 512, (h % 2) * 512 + 512)
                    kp = psc.tile([128, 512], f32, tag="sc", name=f"kp{w}_{h}")
                    for cc in range(NCC):
                        nc.tensor.matmul(kp[0:nq, :], wqk_c[:, cc * 128:cc * 128 + nq],
                                         xh[w * 2 + h // 2][cc][:, hsl],
                                         start=(cc == 0), stop=(cc == NCC - 1))
                    # k parity eviction: 4 j-tiles t0..t3 in this kp
                    t0 = w * 16 + h * 4
                    for d in range(4):
                        t = t0 + d
                        g = t // 2
                        pb = 64 * (t % 2)
                        nc.scalar.activation(
                            k2[pb:pb + 64, g * JT:(g + 1) * JT],
                            kp[0:64, d * JT:(d + 1) * JT],
                            AF.Identity, bias=bias_c[pb:pb + 64, 0:1])
                    if w == 0:
                        jsl = slice(h * 512, (h + 1) * 512)
                        for pb in (0, 64):
                            nc.scalar.activation(
                                q2[pb:pb + 64, jsl], kp[CQ:2 * CQ, :],
                                AF.Identity, bias=bias_c[pb:pb + 64, 1:2])

            emit_qk(0)

            # ---------------- attention + output projection ----------------
            prev = None  # (it, ysb[4], recipB) of the previous i-tile
            pend = None  # (yps, dnmP, dnmV, it) awaiting boundary emission

            def emit_outproj(itp, co, ysb, recipB):
                ipsl = slice(itp * ITILE, (itp + 1) * ITILE)
                op = psc.tile([128, ITILE], f32, tag="sc", name=f"op{itp}_{co}")
                for ci in range(NCC):
                    nc.tensor.matmul(
                        op[:], wvt_c[:, ci * C + co * 128:ci * C + (co + 1) * 128],
                        ysb[ci][:], start=(ci == 0), stop=(ci == NCC - 1))
                ou = opool.tile([128, ITILE], f32, tag="ot", name=f"ou{itp}_{co}")
                nc.vector.tensor_mul(ou[:], op[:], recipB[:])
                ob = opool.tile([128, ITILE], f32, tag="ob", name=f"ob{itp}_{co}")
                nc.scalar.activation(ob[:], ou[:], AF.Identity,
                                     bias=bias_c[:, 2 + co:3 + co])
                qe = nc.sync if co % 2 == 0 else nc.scalar
                qe.dma_start(out_d[co * 128:(co + 1) * 128, ipsl], ob[:])

            for it in range(NITILES):
                isl = slice(it * ITILE, (it + 1) * ITILE)
                yps = [py.tile([128, ITILE], f32, tag="yp", name=f"yp{it}_{i}")
                       for i in range(NCC)]
                dnm = dpool.tile([128, ITILE], f32r, tag="dn", name=f"dn{it}")
                ets = {}

                def emit_pair(p, isl=isl, ets=ets, it=it):
                    gsl = slice(p * JT, (p + 1) * JT)
                    for half, pb in ((0, 0), (1, 64)):
                        j = 2 * p + half
                        sp = psc.tile([JT, ITILE], f32, tag="sc",
                                      name=f"sc{it}_{j}")
                        nc.tensor.matmul(sp[:], k2[pb:pb + 64, gsl],
                                         q2[pb:pb + 64, isl],
                                         start=True, stop=True,
                                         tile_position=(pb, 0))
                        et = epool.tile([JT, ITILE], bf16, tag="exp",
                                        name=f"et{it}_{j}")
                        nc.scalar.activation(et[:], sp[:], AF.Exp)
                        ets[j] = et

                emit_pair(0)

                if pend is not None:
                    # previous i-tile: accumulator eviction first (frees the
                    # PSUM banks for this tile's AV), then the deferred last
                    # denominator add, broadcast matmul and reciprocal.
                    pyps, pdnm, pel, pit = pend
                    ysb = [ypool.tile([128, ITILE], bf16, tag="y",
                                      name=f"y{pit}_{i}") for i in range(NCC)]
                    for cc in range(NCC):
                        nc.vector.tensor_copy(ysb[cc][:], pyps[cc][:])
                    nc.vector.tensor_add(pdnm[:], pdnm[:], pel[:])
                    dB = psc.tile([128, ITILE], f32, tag="sc", name=f"dB{pit}")
                    nc.tensor.matmul(dB[:], ones_sq[:], pdnm[:],
                                     start=True, stop=True)
                    recipB = rpool.tile([128, ITILE], f32, tag="recipB",
                                        name=f"rB{pit}")
                    nc.vector.reciprocal_approx_fast(out=recipB[:], in_=dB[:])
                    prev = (pit, ysb, recipB)
                    pend = None

                for j in range(NJT):
                    et = ets.pop(j)
                    if j == 0:
                        nc.vector.tensor_copy(dnm[:], et[:])
                    elif j < NJT - 1:
                        nc.vector.tensor_add(dnm[:], dnm[:], et[:])
                    else:
                        et_last = et  # deferred: added after the yps eviction
                    if j % 2 == 0 and j + 2 < NJT:
                        emit_pair(j // 2 + 1)
                    xtt = xtw[j // 4]
                    for cc in range(NCC):
                        nc.tensor.matmul(
                            yps[cc][:],
                            xtt[:, (j % 4) * C + cc * 128:(j % 4) * C + (cc + 1) * 128],
                            et[:], start=(j == 0), stop=(j == NJT - 1))
                    if it == 0 and j == 10:
                        emit_qk(1, hs=(0, 2))
                    if it == 0 and j == 14:
                        emit_qk(1, hs=(2, 4))
                    if prev is not None and j in (4, 8, 12, 16):
                        emit_outproj(prev[0], (j - 4) // 4, prev[1], prev[2])

                pend = (yps, dnm, et_last, it)

            # drain: last i-tile boundary + its 4 output projections
            pyps, pdnm, pel, pit = pend
            nc.vector.tensor_add(pdnm[:], pdnm[:], pel[:])
            dB = psc.tile([128, ITILE], f32, tag="sc", name=f"dB{pit}")
            nc.tensor.matmul(dB[:], ones_sq[:], pdnm[:], start=True, stop=True)
            ysb = [ypool.tile([128, ITILE], bf16, tag="y", name=f"y{pit}_{i}")
                   for i in range(NCC)]
            for cc in range(NCC):
                if cc % 2 == 0:
                    nc.vector.tensor_copy(ysb[cc][:], pyps[cc][:])
                else:
                    nc.scalar.activation(ysb[cc][:], pyps[cc][:], AF.Copy)
            recipB = rpool.tile([128, ITILE], f32, tag="recipB", name=f"rB{pit}")
            nc.vector.reciprocal_approx_fast(out=recipB[:], in_=dB[:])
            for co in range(NCC):
                emit_outproj(pit, co, ysb, recipB)

    nc.compile()
    return nc, run_bass_kernel_spmd


def kernel(x, Wq, bq, Wk, bk, Wv, bv, alpha, trace=False, trace_kwargs=None):
    if "nc" not in _cache:
        _cache["nc"] = _build()
    nc, run_spmd = _cache["nc"]

    x = np.ascontiguousarray(np.asarray(x, dtype=np.float32)).reshape(B, C, HW)
    a = float(np.asarray(alpha, np.float32).reshape(-1)[0])
    wqkt = np.concatenate(
        [np.asarray(Wk, np.float32).T, np.asarray(Wq, np.float32).T],
        axis=1).astype(np.float16)  # [C, 128]
    wqk_pack = np.ascontiguousarray(np.concatenate(
        [wqkt[cc * 128:(cc + 1) * 128, :] for cc in range(NCC)], axis=1))
    wvt = (np.asarray(Wv, np.float32).T * a).astype(ml_dtypes.bfloat16)  # [C, C]
    wvt_pack = np.ascontiguousarray(np.concatenate(
        [wvt[cc * 128:(cc + 1) * 128, :] for cc in range(NCC)], axis=1))
    bias = np.zeros((128, 6), np.float32)
    bias[0:64, 0] = np.asarray(bk, np.float32)
    bias[64:128, 0] = np.asarray(bk, np.float32)
    bias[0:64, 1] = np.asarray(bq, np.float32)
    bias[64:128, 1] = np.asarray(bq, np.float32)
    bvf = np.asarray(bv, np.float32) * a
    for cc in range(NCC):
        bias[:, 2 + cc] = bvf[cc * 128:(cc + 1) * 128]

    in_maps = []
    for core in range(NCORES):
        b, ih = core // 2, core % 2
        xb = x[b]
        if ih:
            xb = np.ascontiguousarray(np.concatenate([xb[:, IH:], xb[:, :IH]], axis=1))
        xtp = np.ascontiguousarray(
            xb.T.astype(ml_dtypes.bfloat16).reshape(NJT, 128, C)
            .transpose(1, 0, 2).reshape(128, NJT * C))
        in_maps.append({"x": xb.astype(np.float16), "xt": xtp,
                        "wqk": wqk_pack, "wvt": wvt_pack, "bias": bias})

    kwargs = {}
    if trace:
        kwargs["trace"] = True
        kwargs.update(trace_kwargs or {})
    res = run_spmd(nc, in_maps, list(range(NCORES)), **kwargs)

    out = np.empty((B, C, HW), dtype=np.float32)
    for core in range(NCORES):
        b, ih = core // 2, core % 2
        out[b][:, ih * IH:(ih + 1) * IH] = res.results[core]["out"]
    if trace:
        return out.reshape(B, C, 64, 64), res
    return out.reshape(B, C, 64, 64)
